# revision 1
# baseline (speedup 1.0000x reference)
"""Trainium2 Bass kernel for nn_AttentionBlock (GroupNorm + 8-head attention
block on [8, 512, 32, 32], residual).

Sharding: pure data-parallel over batch B=8 across the 8 NeuronCores — one
batch element per core, weights replicated, zero collectives.

Per-core dataflow (one batch element, x as [C=512, HW=1024] f32):
  1. GroupNorm(32 groups of 16 channels): raw sums via DVE reduce + ACT square
     accum, cross-partition group combine + expand via tiny matmuls with
     host-provided selector constants, then h = a*x + d on ACT (bf16 out).
  2. proj_in: q, k [512, 1024] (out-channels on partitions) and vT [1024, 520]
     (pixels on partitions; per head 64 v-columns + a constant ones column)
     via bf16 matmuls against host-pre-transposed w_inT.
  3. Per head pair: eT = exp(scale * k_h^T q_h) — the two heads' K=64 logits
     matmuls run concurrently in the two row-halves of the PE array
     (tile_position auto from base partitions); exp on ACT (bf16 out).
     out2[65, 1024] = [vT_h | ones]^T @ eT accumulated over the 8 k-tiles:
     rows 0..63 = unnormalized attention out, row 64 = softmax denominator.
     Reciprocal of the denominator row, then a DRAM round-trip DMA broadcast
     (stride-0 partition dim) to 64 partitions; attn_h = out2*recip + b_v.
  4. proj_out + bias + residual: matmul against host-pre-transposed w_outT,
     then one fused scalar_tensor_tensor: (psum + b_out) + x -> out f32.
"""
import sys

sys.path.insert(0, "/opt/trn_rl_repo")

import numpy as np
import ml_dtypes

import concourse.bass as bass
import concourse.bacc as bacc
import concourse.tile as tile
from concourse import mybir
from concourse.bass_utils import run_bass_kernel_spmd

F32 = mybir.dt.float32
BF16 = mybir.dt.bfloat16
ADD = mybir.AluOpType.add
MULT = mybir.AluOpType.mult

B, C, H, W = 8, 512, 32, 32
HW = H * W       # 1024
NG = 32          # groups
GS = C // NG     # 16 channels per group
NH = 8           # heads
HD = 64          # head dim
HID = NH * HD    # 512
EPS = 1e-6
SCALE = 1.0 / float(np.sqrt(HD))  # 0.125
CT = C // 128    # 4 channel partition-tiles
PT = HW // 128   # 8 pixel partition-tiles
GN_INV = 1.0 / (GS * HW)          # 1/16384


def build_graph():
    nc = bacc.Bacc("TRN2", num_devices=8)

    x_ext = nc.declare_dram_parameter("x", [C, HW], F32, isOutput=False)
    w_inT_ext = nc.declare_dram_parameter("w_inT", [C, 3 * HID], BF16, isOutput=False)
    w_outT_ext = nc.declare_dram_parameter("w_outT", [HID, C], BF16, isOutput=False)
    b_in_ext = nc.declare_dram_parameter("b_in_pm", [128, 12], F32, isOutput=False)
    b_v_ext = nc.declare_dram_parameter("b_v_pm", [HD, NH], F32, isOutput=False)
    b_out_ext = nc.declare_dram_parameter("b_out_pm", [128, CT], F32, isOutput=False)
    gamma_ext = nc.declare_dram_parameter("gamma_pm", [128, CT], F32, isOutput=False)
    beta_ext = nc.declare_dram_parameter("beta_pm", [128, CT], F32, isOutput=False)
    sel_ext = nc.declare_dram_parameter("gn_sel", [128, CT, NG], F32, isOutput=False)
    selT_ext = nc.declare_dram_parameter("gn_selT", [NG, CT, 128], F32, isOutput=False)
    out_ext = nc.declare_dram_parameter("out", [C, HW], F32, isOutput=True)

    recip_dram = nc.dram_tensor("recip_scratch", [NH, HW], F32)

    with tile.TileContext(nc) as tc:
        with (
            tc.tile_pool(name="const", bufs=1) as const,
            tc.tile_pool(name="big", bufs=1) as big,
            tc.tile_pool(name="eT", bufs=1) as eTp,
            tc.tile_pool(name="small", bufs=2) as small,
        ):
            # ---------- loads ----------
            x_sb = [big.tile([128, HW], F32, tag=f"x{t}", name=f"x{t}") for t in range(CT)]
            for t in range(CT):
                nc.gpsimd.dma_start(out=x_sb[t], in_=x_ext[128 * t:128 * (t + 1), :])
            w_inT_sb = [big.tile([128, 3 * HID], BF16, tag=f"wi{t}", name=f"wi{t}") for t in range(CT)]
            for t in range(CT):
                nc.gpsimd.dma_start(out=w_inT_sb[t],
                                    in_=w_inT_ext[128 * t:128 * (t + 1), :])
            w_outT_sb = [big.tile([128, C], BF16, tag=f"wo{t}", name=f"wo{t}") for t in range(CT)]
            for t in range(CT):
                nc.gpsimd.dma_start(out=w_outT_sb[t],
                                    in_=w_outT_ext[128 * t:128 * (t + 1), :])
            b_in_sb = const.tile([128, 12], F32)
            nc.gpsimd.dma_start(out=b_in_sb, in_=b_in_ext[:, :])
            b_v_sb = const.tile([HD, NH], F32)
            nc.gpsimd.dma_start(out=b_v_sb, in_=b_v_ext[:, :])
            b_out_sb = const.tile([128, CT], F32)
            nc.gpsimd.dma_start(out=b_out_sb, in_=b_out_ext[:, :])
            gamma_sb = const.tile([128, CT], F32)
            nc.gpsimd.dma_start(out=gamma_sb, in_=gamma_ext[:, :])
            beta_sb = const.tile([128, CT], F32)
            nc.gpsimd.dma_start(out=beta_sb, in_=beta_ext[:, :])
            sel_sb = const.tile([128, CT, NG], F32)
            nc.gpsimd.dma_start(out=sel_sb, in_=sel_ext[:, :, :])
            selT_sb = const.tile([NG, CT, 128], F32)
            nc.gpsimd.dma_start(out=selT_sb, in_=selT_ext[:, :, :])

            # ---------- groupnorm ----------
            with tc.tile_pool(name="ps_gn", bufs=2, space="PSUM") as ps_gn:
                stats = [small.tile([128, 2], F32, tag=f"st{t}", bufs=1, name=f"st{t}")
                         for t in range(CT)]
                sq_scratch = small.tile([128, HW], F32, tag="sqs", bufs=1)
                for t in range(CT):
                    nc.vector.reduce_sum(stats[t][:, 0:1], x_sb[t][:, :],
                                         axis=mybir.AxisListType.X)
                    nc.scalar.activation(out=sq_scratch, in_=x_sb[t][:, :],
                                         func=mybir.ActivationFunctionType.Square,
                                         accum_out=stats[t][:, 1:2])
                gpsum = ps_gn.tile([NG, 2], F32, tag="gps")
                for t in range(CT):
                    nc.tensor.matmul(gpsum[:, :], lhsT=sel_sb[:, t, :],
                                     rhs=stats[t][:, :],
                                     start=(t == 0), stop=(t == CT - 1))
                # grp cols: 0 rstd, 1 mean*rstd, 2 mean, 3 E[x^2] (later scratch)
                grp = small.tile([NG, 4], F32, tag="grp", bufs=1)
                eps_sb = small.tile([NG, 1], F32, tag="eps_c", bufs=1)
                nc.vector.memset(eps_sb, float(EPS))
                nc.vector.tensor_scalar_mul(grp[:, 2:4], gpsum[:, 0:2], GN_INV)
                nc.vector.tensor_mul(grp[:, 0:1], grp[:, 2:3], grp[:, 2:3])  # mean^2
                nc.vector.tensor_sub(grp[:, 0:1], grp[:, 3:4], grp[:, 0:1])  # var
                nc.scalar.activation(out=grp[:, 0:1], in_=grp[:, 0:1],
                                     func=mybir.ActivationFunctionType.Sqrt,
                                     bias=eps_sb[:, :], scale=1.0)
                nc.vector.reciprocal(out=grp[:, 0:1], in_=grp[:, 0:1])  # rstd
                nc.vector.tensor_mul(grp[:, 1:2], grp[:, 2:3], grp[:, 0:1])
                ga = [small.tile([128, 1], F32, tag=f"ga{t}", bufs=1, name=f"ga{t}")
                      for t in range(CT)]
                gd = [small.tile([128, 1], F32, tag=f"gd{t}", bufs=1, name=f"gd{t}")
                      for t in range(CT)]
                for t in range(CT):
                    epsum = ps_gn.tile([128, 2], F32, tag="eps")
                    nc.tensor.matmul(epsum[:, :], lhsT=selT_sb[:, t, :],
                                     rhs=grp[:, 0:2], start=True, stop=True)
                    nc.vector.tensor_mul(ga[t][:, :], gamma_sb[:, t:t + 1],
                                         epsum[:, 0:1])
                    # d = beta - gamma * (mean*rstd)
                    nc.vector.tensor_mul(gd[t][:, :], gamma_sb[:, t:t + 1],
                                         epsum[:, 1:2])
                    nc.vector.tensor_sub(gd[t][:, :], beta_sb[:, t:t + 1],
                                         gd[t][:, :])
                h_sb = [big.tile([128, HW], BF16, tag=f"h{t}", name=f"h{t}") for t in range(CT)]
                for t in range(CT):
                    nc.scalar.activation(out=h_sb[t], in_=x_sb[t][:, :],
                                         func=mybir.ActivationFunctionType.Identity,
                                         bias=gd[t][:, :], scale=ga[t][:, :])

            # ---------- proj_in ----------
            q_sb = [big.tile([128, HW], BF16, tag=f"q{m}", name=f"q{m}") for m in range(4)]
            k_sb = [big.tile([128, HW], BF16, tag=f"k{m}", name=f"k{m}") for m in range(4)]
            vT_sb = [big.tile([128, NH, HD + 1], BF16, tag=f"vT{p}", name=f"vT{p}")
                     for p in range(PT)]
            with tc.tile_pool(name="ps_pin", bufs=4, space="PSUM") as ps_pin:
                for dest, off in ((q_sb, 0), (k_sb, HID)):
                    for m in range(4):
                        bcol = (off + 128 * m) // 128
                        for n in range(2):
                            pp = ps_pin.tile([128, 512], F32, tag="pp")
                            for t in range(CT):
                                nc.tensor.matmul(
                                    pp[:, :],
                                    lhsT=w_inT_sb[t][:, off + 128 * m:
                                                     off + 128 * (m + 1)],
                                    rhs=h_sb[t][:, 512 * n:512 * (n + 1)],
                                    start=(t == 0), stop=(t == CT - 1))
                            nc.vector.tensor_scalar(
                                out=dest[m][:, 512 * n:512 * (n + 1)], in0=pp[:, :],
                                scalar1=b_in_sb[:, bcol:bcol + 1], scalar2=None,
                                op0=ADD)
                for p in range(PT):
                    nc.vector.memset(vT_sb[p], 1.0)
                for p in range(PT):
                    pp = ps_pin.tile([128, 512], F32, tag="pp")
                    for t in range(CT):
                        nc.tensor.matmul(
                            pp[:, :],
                            lhsT=h_sb[t][:, 128 * p:128 * (p + 1)],
                            rhs=w_inT_sb[t][:, 2 * HID:3 * HID],
                            start=(t == 0), stop=(t == CT - 1))
                    nc.vector.tensor_copy(
                        out=vT_sb[p][:, :, 0:HD],
                        in_=pp[:, :].rearrange("a (nh c) -> a nh c", nh=NH))

            # ---------- attention ----------
            attn_sb = [big.tile([128, HW], BF16, tag=f"at{i}", name=f"at{i}") for i in range(4)]
            with (
                tc.tile_pool(name="ps_log", bufs=2, space="PSUM") as ps_log,
                tc.tile_pool(name="ps_o2", bufs=2, space="PSUM") as ps_o2,
            ):
                eT_all = {}

                def emit_logits_exp(hp):
                    eTs = []
                    for sub in range(2):
                        eTs.append([eTp.tile([128, HW], BF16, bufs=2,
                                             tag=f"eT{sub}_{p}",
                                             name=f"eT{hp}_{sub}_{p}")
                                    for p in range(PT)])
                    eT_all[hp] = eTs
                    for p in range(PT):
                        pls = []
                        for sub in range(2):
                            lo, hi = 64 * sub, 64 * (sub + 1)
                            pl = ps_log.tile([128, HW], F32, tag="plog",
                                             name=f"pl{hp}_{sub}_{p}")
                            for n in range(2):
                                nc.tensor.matmul(
                                    pl[:, 512 * n:512 * (n + 1)],
                                    lhsT=k_sb[hp][lo:hi, 128 * p:128 * (p + 1)],
                                    rhs=q_sb[hp][lo:hi, 512 * n:512 * (n + 1)],
                                    start=True, stop=True)
                            pls.append(pl)
                        for sub in range(2):
                            nc.scalar.activation(
                                out=eTs[sub][p], in_=pls[sub][:, :],
                                func=mybir.ActivationFunctionType.Exp,
                                scale=SCALE)

                def emit_out2_norm(hp):
                    eTs = eT_all.pop(hp)
                    for sub in range(2):
                        head = 2 * hp + sub
                        eT = eTs[sub]
                        po = ps_o2.tile([HD + 1, HW], F32, tag="po2",
                                        name=f"po{head}")
                        for p in range(PT):
                            for n in range(2):
                                nc.tensor.matmul(
                                    po[:, 512 * n:512 * (n + 1)],
                                    lhsT=vT_sb[p][:, head, :],
                                    rhs=eT[p][:, 512 * n:512 * (n + 1)],
                                    start=(p == 0), stop=(p == PT - 1))
                        # early-evict unnormalized out (frees the PSUM slot)
                        attn_u = small.tile([64, HW], BF16, tag="attnu",
                                            bufs=4, name=f"attnu{head}")
                        nc.vector.tensor_copy(out=attn_u, in_=po[0:HD, :])
                        # fast reciprocal of denom row (PSUM p64 -> SBUF p64)
                        rrow = small.tile([HD + 1, HW], F32, tag="rrow",
                                          name=f"rrow{head}")
                        nc.vector.reciprocal(
                            out=rrow[HD:HD + 1, :], in_=po[HD:HD + 1, :])
                        nc.sync.dma_start(out=recip_dram[head:head + 1, :],
                                          in_=rrow[HD:HD + 1, :])
                        rb = small.tile([64, HW], F32, tag="rb",
                                        name=f"rb{head}")
                        bcast_ap = bass.AP(
                            tensor=recip_dram[:, :].tensor,
                            offset=head * HW,
                            ap=[[0, 64], [1, HW]])
                        nc.sync.dma_start(out=rb, in_=bcast_ap)
                        tmp = small.tile([64, HW], BF16, tag="atmp",
                                         name=f"atmp{head}")
                        nc.vector.tensor_mul(tmp[:, :], attn_u[:, :], rb[:, :])
                        if sub == 0:
                            nc.vector.tensor_scalar(
                                out=attn_sb[hp][0:64, :], in0=tmp[:, :],
                                scalar1=b_v_sb[:, head:head + 1],
                                scalar2=None, op0=ADD)
                        else:
                            tmp2 = small.tile([64, HW], BF16, tag="atmp2",
                                              name=f"atmp2{head}")
                            nc.vector.tensor_scalar(
                                out=tmp2[:, :], in0=tmp[:, :],
                                scalar1=b_v_sb[:, head:head + 1],
                                scalar2=None, op0=ADD)
                            nc.sync.dma_start(out=attn_sb[hp][64:128, :],
                                              in_=tmp2)

                # software-pipelined: logits/exp of pair hp overlap
                # out2/normalize of pair hp-1
                for step in range(5):
                    if step < 4:
                        emit_logits_exp(step)
                    if step >= 1:
                        emit_out2_norm(step - 1)

            # ---------- proj_out + bias + residual ----------
            with tc.tile_pool(name="ps_pout", bufs=4, space="PSUM") as ps_pout:
                for m in range(4):
                    for n in range(2):
                        pp = ps_pout.tile([128, 512], F32, tag="pp")
                        for t in range(CT):
                            nc.tensor.matmul(
                                pp[:, :],
                                lhsT=w_outT_sb[t][:, 128 * m:128 * (m + 1)],
                                rhs=attn_sb[t][:, 512 * n:512 * (n + 1)],
                                start=(t == 0), stop=(t == CT - 1))
                        o_sb = small.tile([128, 512], F32, tag="osb")
                        nc.vector.scalar_tensor_tensor(
                            out=o_sb, in0=pp[:, :], scalar=b_out_sb[:, m:m + 1],
                            in1=x_sb[m][:, 512 * n:512 * (n + 1)],
                            op0=ADD, op1=ADD)
                        nc.sync.dma_start(
                            out=out_ext[128 * m:128 * (m + 1),
                                        512 * n:512 * (n + 1)],
                            in_=o_sb)
    return nc


def _install_ntff_hook():
    """The agent image's antenv lacks axon_hooks; synthesize it so
    run_bass_kernel_spmd(trace=True) can reach the NTFF profiler."""
    import types
    if "antenv.axon_hooks" in sys.modules:
        return
    mod = types.ModuleType("antenv.axon_hooks")
    mod._hook = None

    def set_axon_ntff_profile_hook(hook):
        mod._hook = hook

    def get_axon_ntff_profile_hook():
        return mod._hook

    mod.set_axon_ntff_profile_hook = set_axon_ntff_profile_hook
    mod.get_axon_ntff_profile_hook = get_axon_ntff_profile_hook
    sys.modules["antenv.axon_hooks"] = mod
    try:
        from trn_agent_boot.trn_boot import _ntff_profile_via_ctypes
        hook = _ntff_profile_via_ctypes("/opt/axon/libaxon_pjrt.so")
        if hook is not None:
            set_axon_ntff_profile_hook(hook)
    except Exception as e:  # degrade to no tracing
        print("ntff hook setup failed:", e)


_COMPILED = None


def _get_compiled():
    global _COMPILED
    if _COMPILED is None:
        nc = build_graph()
        nc.compile()
        _COMPILED = nc
    return _COMPILED


def _make_consts():
    sel = np.zeros((128, CT, NG), dtype=np.float32)
    selT = np.zeros((NG, CT, 128), dtype=np.float32)
    for t in range(CT):
        for p in range(128):
            g = 8 * t + p // GS
            sel[p, t, g] = 1.0
            selT[g, t, p] = 1.0
    return sel, selT


def _pm(v, cols):
    """[cols*128] vector -> partition-major [128, cols]."""
    return np.ascontiguousarray(v.reshape(cols, 128).T)


def kernel(x, gamma, beta, w_in, b_in, w_out, b_out, _trace=False):
    x = np.asarray(x, dtype=np.float32)
    gamma = np.asarray(gamma, dtype=np.float32)
    beta = np.asarray(beta, dtype=np.float32)
    w_in = np.asarray(w_in, dtype=np.float32)
    b_in = np.asarray(b_in, dtype=np.float32)
    w_out = np.asarray(w_out, dtype=np.float32)
    b_out = np.asarray(b_out, dtype=np.float32)

    w_inT = np.ascontiguousarray(w_in.T).astype(ml_dtypes.bfloat16)
    w_outT = np.ascontiguousarray(w_out.T).astype(ml_dtypes.bfloat16)
    sel, selT = _make_consts()
    b_v = b_in[2 * HID:3 * HID]
    b_v_pm = np.ascontiguousarray(b_v.reshape(NH, HD).T)  # [64, 8]
    common = {
        "w_inT": w_inT,
        "w_outT": w_outT,
        "b_in_pm": _pm(b_in, 12),
        "b_v_pm": b_v_pm,
        "b_out_pm": _pm(b_out, CT),
        "gamma_pm": _pm(gamma, CT),
        "beta_pm": _pm(beta, CT),
        "gn_sel": sel,
        "gn_selT": selT,
    }
    in_maps = []
    for b in range(B):
        m = dict(common)
        m["x"] = np.ascontiguousarray(x[b].reshape(C, HW))
        in_maps.append(m)

    if _trace:
        _install_ntff_hook()
    nc = _get_compiled()
    res = run_bass_kernel_spmd(nc, in_maps, core_ids=list(range(B)),
                               trace=_trace)
    out = np.stack([np.asarray(res.results[b]["out"]).reshape(C, H, W)
                    for b in range(B)])
    if _trace:
        return out, res
    return out


if __name__ == "__main__":
    rng = np.random.default_rng(0)
    inputs = {
        "x": rng.standard_normal((B, C, H, W), dtype=np.float32),
        "gamma": np.ones(C, dtype=np.float32),
        "beta": np.zeros(C, dtype=np.float32),
        "w_in": (rng.standard_normal((3 * HID, C), dtype=np.float32)
                 / np.sqrt(C)),
        "b_in": np.zeros(3 * HID, dtype=np.float32),
        "w_out": (rng.standard_normal((C, HID), dtype=np.float32)
                  / np.sqrt(HID)),
        "b_out": np.zeros(C, dtype=np.float32),
    }
    out = kernel(**inputs)
    print("kernel ran, out shape", out.shape)



# revision 8
# speedup vs baseline: 1.3457x; 1.3457x over previous
"""Trainium2 Bass kernel for nn_AttentionBlock (GroupNorm + 8-head attention
block on [8, 512, 32, 32], residual).

Sharding: pure data-parallel over batch B=8 across the 8 NeuronCores — one
batch element per core, weights replicated, zero collectives.

v2 design (ACT-exp is the wall at ~73us; everything else hides under it):
  - gamma/beta folded into host-preprocessed weights: w_inT_g = w_in.T * gamma,
    biases b_eff = b_in + w_in @ beta.  Device GN = (x - mean) * rstd only,
    with rstd = exp(-0.5*ln(var+eps)) so the whole kernel uses ONE ACT table
    set (natural_log_exp: ln, exp, square, identity).
  - x shipped as bf16 (halves input DMA); per-channel-tile GN pipelined so
    proj_in matmuls start as soon as h tiles exist.
  - v-bias and out-bias deferred: c0 = w_out @ b_v_eff + b_out added at the
    final residual step (softmax weights sum to 1).
  - PE warm-up junk matmuls at start (HAM clock gate: 1.2 -> 2.4 GHz after
    ~3.4us of sustained busy).
  - attention: q,k projected first, logits+exp of pair 0 launched before the
    v projection; pairs software-pipelined; out2 uses a ones-column (M=65) to
    get softmax denominators for free; denominators evicted per-pair,
    reciprocal_approx_fast, DRAM-round-trip broadcast, normalize fused into
    the PSUM eviction (one DVE tensor_tensor per head).
"""
import sys

sys.path.insert(0, "/opt/trn_rl_repo")

import numpy as np
import ml_dtypes

import concourse.bass as bass
import concourse.bacc as bacc
import concourse.tile as tile
from concourse import mybir
from concourse.bass_utils import run_bass_kernel_spmd

F32 = mybir.dt.float32
BF16 = mybir.dt.bfloat16
ADD = mybir.AluOpType.add
SUB = mybir.AluOpType.subtract
MULT = mybir.AluOpType.mult
AF = mybir.ActivationFunctionType

B, C, H, W = 8, 512, 32, 32
HW = H * W       # 1024
NG = 32          # groups
GS = C // NG     # 16 channels per group
NH = 8           # heads
HD = 64          # head dim
HID = NH * HD    # 512
EPS = 1e-6
SCALE = 1.0 / float(np.sqrt(HD))  # 0.125
CT = C // 128    # 4 channel partition-tiles
PT = HW // 128   # 8 pixel partition-tiles
GN_INV = 1.0 / (GS * HW)          # 1/16384
N_JUNK = 14      # PE warm-up matmuls


def build_graph(debug=False):
    nc = bacc.Bacc("TRN2", num_devices=8)

    x_ext = nc.declare_dram_parameter("xbf", [C, HW], BF16, isOutput=False)
    w_inT_ext = nc.declare_dram_parameter("w_inT_g", [C, 3 * HID], BF16, isOutput=False)
    w_outT_ext = nc.declare_dram_parameter("w_outT", [HID, C], BF16, isOutput=False)
    b_q_ext = nc.declare_dram_parameter("b_q_pm", [128, CT], F32, isOutput=False)
    b_k_ext = nc.declare_dram_parameter("b_k_pm", [128, CT], F32, isOutput=False)
    c0_ext = nc.declare_dram_parameter("c0_pm", [128, CT], F32, isOutput=False)
    sel_ext = nc.declare_dram_parameter("gn_sel8", [128, 8], F32, isOutput=False)
    selT_ext = nc.declare_dram_parameter("gn_selT8", [8, 128], F32, isOutput=False)
    out_ext = nc.declare_dram_parameter("out", [C, HW], F32, isOutput=True)

    recip_dram = nc.dram_tensor("recip_scratch", [4, 2 * HW], F32)
    junk_dram = nc.dram_tensor("junk_scratch", [1, 512], F32)
    dbg = {}
    if debug:
        dbg["h0"] = nc.declare_dram_parameter("dbg_h0", [128, HW], BF16, isOutput=True)
        dbg["q0"] = nc.declare_dram_parameter("dbg_q0", [128, HW], BF16, isOutput=True)
        dbg["k0"] = nc.declare_dram_parameter("dbg_k0", [128, HW], BF16, isOutput=True)
        dbg["vT0"] = nc.declare_dram_parameter("dbg_vT0", [128, NH * (HD + 1)], BF16, isOutput=True)
        dbg["eT000"] = nc.declare_dram_parameter("dbg_eT000", [128, HW], BF16, isOutput=True)
        dbg["den0"] = nc.declare_dram_parameter("dbg_den0", [1, 2 * HW], F32, isOutput=True)
        dbg["rr0"] = nc.declare_dram_parameter("dbg_rr0", [1, 2 * HW], F32, isOutput=True)
        dbg["rb00"] = nc.declare_dram_parameter("dbg_rb00", [64, HW], F32, isOutput=True)
        dbg["attn0"] = nc.declare_dram_parameter("dbg_attn0", [128, HW], BF16, isOutput=True)

    with tile.TileContext(nc) as tc:
        with (
            tc.tile_pool(name="const", bufs=1) as const,
            tc.tile_pool(name="big", bufs=1) as big,
            tc.tile_pool(name="eT", bufs=1) as eTp,
            tc.tile_pool(name="small", bufs=2) as small,
            tc.tile_pool(name="pl_pool", bufs=2, space="PSUM") as pl_pool,
        ):
            pin_cm = tc.tile_pool(name="pin", bufs=1, space="PSUM")
            pin = pin_cm.__enter__()
            # ---------- tiny on-chip constants (no DMA) ----------
            warm_sb = small.tile([128, 512], BF16, tag="warm", bufs=1)
            nc.vector.memset(warm_sb, 0.25)
            eps_sb = small.tile([8, 1], F32, tag="eps_c", bufs=1)
            nc.vector.memset(eps_sb, float(EPS))
            # preload the natural_log_exp act table set ASAP
            dummy_sb = small.tile([1, 1], F32, tag="dummy", bufs=1)
            nc.scalar.activation(out=dummy_sb, in_=eps_sb[0:1, 0:1], func=AF.Ln,
                                 bias=eps_sb[0:1, :], scale=1.0)

            # ---------- input DMAs, issue spread across engines ----------
            x_sb = [big.tile([128, HW], BF16, tag=f"x{t}", name=f"x{t}")
                    for t in range(CT)]
            for t in range(CT):
                nc.sync.dma_start(out=x_sb[t], in_=x_ext[128 * t:128 * (t + 1), :])
            sel_sb = const.tile([128, 8], F32)
            nc.gpsimd.dma_start(out=sel_sb, in_=sel_ext[:, :])
            selT_sb = const.tile([8, 128], F32)
            nc.gpsimd.dma_start(out=selT_sb, in_=selT_ext[:, :])
            w_inT_sb = [big.tile([128, 3 * HID], BF16, tag=f"wi{t}", name=f"wi{t}")
                        for t in range(CT)]
            for t in range(CT):
                nc.gpsimd.dma_start(out=w_inT_sb[t],
                                    in_=w_inT_ext[128 * t:128 * (t + 1), :])
            b_q_sb = const.tile([128, CT], F32)
            nc.gpsimd.dma_start(out=b_q_sb, in_=b_q_ext[:, :])
            b_k_sb = const.tile([128, CT], F32)
            nc.gpsimd.dma_start(out=b_k_sb, in_=b_k_ext[:, :])
            c0_sb = const.tile([128, CT], F32)
            nc.gpsimd.dma_start(out=c0_sb, in_=c0_ext[:, :])
            w_outT_sb = [big.tile([128, C], BF16, tag=f"wo{t}", name=f"wo{t}")
                         for t in range(CT)]
            for t in range(CT):
                nc.sync.dma_start(out=w_outT_sb[t],
                                  in_=w_outT_ext[128 * t:128 * (t + 1), :])

            # ---------- PE warm-up (HAM un-throttle) ----------
            jp = None
            for j in range(N_JUNK):
                jp = pin.tile([128, 512], F32, tag="pp", bufs=2, name=f"junk{j}")
                nc.tensor.matmul(jp[:, :], lhsT=warm_sb[:, 0:128],
                                 rhs=warm_sb[:, :], start=True, stop=True)
            junk_sb = small.tile([1, 512], F32, tag="junk_s", bufs=1)
            nc.vector.tensor_copy(out=junk_sb, in_=jp[0:1, :])
            nc.sync.dma_start(out=junk_dram[0:1, :], in_=junk_sb)

            # ---------- groupnorm (per 128-channel tile; groups don't cross
            # tiles).  h[t] = x[t]*rstd - mean*rstd, gamma/beta in weights. ----
            h_sb = [big.tile([128, HW], BF16, tag=f"h{t}", name=f"h{t}")
                    for t in range(CT)]
            sq_scratch = [small.tile([128, HW], BF16, tag=f"sqs{t % 2}", bufs=1,
                                     name=f"sqs{t}") for t in range(CT)]
            for t in range(CT):
                stats = small.tile([128, 2], F32, tag=f"st{t}", bufs=1,
                                   name=f"st{t}")
                nc.vector.reduce_sum(stats[:, 0:1], x_sb[t][:, :],
                                     axis=mybir.AxisListType.X)
                nc.scalar.activation(out=sq_scratch[t], in_=x_sb[t][:, :],
                                     func=AF.Square,
                                     accum_out=stats[:, 1:2])
                gpsum = pin.tile([8, 2], F32, tag="gps", bufs=1, name=f"gps{t}")
                nc.tensor.matmul(gpsum[:, :], lhsT=sel_sb[:, :],
                                 rhs=stats[:, :], start=True, stop=True)
                # grp cols: 0 = rstd (later), 1 = mean, 2 = E[x^2], 3 = var
                grp = small.tile([8, 4], F32, tag=f"grp{t}", bufs=1,
                                 name=f"grp{t}")
                nc.vector.tensor_scalar_mul(grp[:, 1:3], gpsum[:, 0:2], GN_INV)
                nc.vector.tensor_mul(grp[:, 3:4], grp[:, 1:2], grp[:, 1:2])
                nc.vector.tensor_sub(grp[:, 3:4], grp[:, 2:3], grp[:, 3:4])
                # rstd = exp(-0.5 * ln(var + eps))  (same act table set as Exp)
                nc.scalar.activation(out=grp[:, 2:3], in_=grp[:, 3:4],
                                     func=AF.Ln, bias=eps_sb[:, :], scale=1.0)
                nc.scalar.activation(out=grp[:, 0:1], in_=grp[:, 2:3],
                                     func=AF.Exp, scale=-0.5)
                nc.vector.tensor_mul(grp[:, 1:2], grp[:, 1:2], grp[:, 0:1])
                epsum = pin.tile([128, 2], F32, tag="eps_ps", bufs=1,
                                 name=f"eps_ps{t}")
                nc.tensor.matmul(epsum[:, :], lhsT=selT_sb[:, :],
                                 rhs=grp[:, 0:2], start=True, stop=True)
                ab = small.tile([128, 2], F32, tag=f"ab{t}", bufs=1,
                                name=f"ab{t}")
                nc.vector.tensor_copy(out=ab, in_=epsum[:, :])
                nc.vector.tensor_scalar(
                    out=h_sb[t], in0=x_sb[t][:, :],
                    scalar1=ab[:, 0:1], scalar2=ab[:, 1:2],
                    op0=MULT, op1=SUB)

            # ---------- proj_in q,k (m=0 first), then logits pair0, then v --
            q_sb = [big.tile([128, HW], BF16, tag=f"q{m}", name=f"q{m}")
                    for m in range(4)]
            k_sb = [big.tile([128, HW], BF16, tag=f"k{m}", name=f"k{m}")
                    for m in range(4)]
            vT_sb = [big.tile([128, NH, HD + 1], BF16, tag=f"vT{p}",
                              name=f"vT{p}") for p in range(PT)]
            for p in range(PT):
                nc.gpsimd.memset(vT_sb[p], 1.0)

            def emit_qk(m):
                for dest, off, bias in ((q_sb, 0, b_q_sb), (k_sb, HID, b_k_sb)):
                    pps = [pin.tile([128, 512], F32, tag="pp", bufs=2,
                                    name=f"pp{'qk'[off > 0]}{m}_{n}")
                           for n in range(2)]
                    for t in range(CT):
                        for n in range(2):
                            nc.tensor.matmul(
                                pps[n][:, :],
                                lhsT=w_inT_sb[t][:, off + 128 * m:
                                                 off + 128 * (m + 1)],
                                rhs=h_sb[t][:, 512 * n:512 * (n + 1)],
                                start=(t == 0), stop=(t == CT - 1))
                    for n in range(2):
                        nc.vector.tensor_scalar(
                            out=dest[m][:, 512 * n:512 * (n + 1)],
                            in0=pps[n][:, :],
                            scalar1=bias[:, m:m + 1], scalar2=None, op0=ADD)

            def emit_v(p):
                pp = pin.tile([128, 512], F32, tag="pp", bufs=2, name=f"ppv{p}")
                for t in range(CT):
                    nc.tensor.matmul(
                        pp[:, :],
                        lhsT=h_sb[t][:, 128 * p:128 * (p + 1)],
                        rhs=w_inT_sb[t][:, 2 * HID:3 * HID],
                        start=(t == 0), stop=(t == CT - 1))
                nc.vector.tensor_copy(
                    out=vT_sb[p][:, :, 0:HD],
                    in_=pp[:, :].rearrange("a (nh c) -> a nh c", nh=NH))

            # attention helpers -------------------------------------------
            attn_sb = [big.tile([128, HW], BF16, tag=f"at{i}", name=f"at{i}")
                       for i in range(4)]
            eT_all = {}

            def emit_logits_exp(hp, p):
                """logits + exp for pair hp, pixel-tile p (2 heads row-paired)."""
                if p == 0:
                    eT_all[hp] = [[eTp.tile([128, HW], BF16, bufs=2,
                                            tag=f"eT{sub}_{pp_}",
                                            name=f"eT{hp}_{sub}_{pp_}")
                                   for pp_ in range(PT)] for sub in range(2)]
                pls = []
                for sub in range(2):
                    pls.append(pl_pool.tile([128, HW], F32, tag="pl",
                                            name=f"pl{hp}_{sub}_{p}"))
                for n in range(2):
                    for sub in range(2):
                        lo, hi = 64 * sub, 64 * (sub + 1)
                        nc.tensor.matmul(
                            pls[sub][:, 512 * n:512 * (n + 1)],
                            lhsT=k_sb[hp][lo:hi, 128 * p:128 * (p + 1)],
                            rhs=q_sb[hp][lo:hi, 512 * n:512 * (n + 1)],
                            start=True, stop=True)
                for sub in range(2):
                    nc.scalar.activation(
                        out=eT_all[hp][sub][p], in_=pls[sub][:, :],
                        func=AF.Exp, scale=SCALE)
                if debug and hp == 0 and p == 0:
                    nc.gpsimd.dma_start(out=dbg["eT000"][:, :],
                                        in_=eT_all[0][0][0])

            def emit_out2_mm(hp, p, po_pair):
                """out2 accumulation step p for both heads of pair hp."""
                for sub in range(2):
                    head = 2 * hp + sub
                    for n in range(2):
                        nc.tensor.matmul(
                            po_pair[sub][:, 512 * n:512 * (n + 1)],
                            lhsT=vT_sb[p][:, head, :],
                            rhs=eT_all[hp][sub][p][:, 512 * n:512 * (n + 1)],
                            start=(p == 0), stop=(p == PT - 1))

            def emit_norm(hp, po_pair):
                """denominator evict + recip + broadcast + fused normalize.
                reciprocal_approx_fast only works at partition base 0, so the
                denominator rows (PSUM partition 64) are evicted to base 0."""
                den = small.tile([1, 2 * HW], F32, tag="den", bufs=1,
                                 name=f"den{hp}")
                for sub in range(2):
                    nc.vector.tensor_copy(
                        out=den[0:1, HW * sub:HW * (sub + 1)],
                        in_=po_pair[sub][64:65, :])
                rr = small.tile([1, 2 * HW], F32, tag="rr", bufs=1,
                                name=f"rr{hp}")
                nc.vector.reciprocal_approx_fast(out=rr, in_=den)
                nc.sync.dma_start(out=recip_dram[hp:hp + 1, :], in_=rr)
                rb = [small.tile([64, HW], F32, tag=f"rb{sub}", bufs=2,
                                 name=f"rb{hp}_{sub}") for sub in range(2)]
                for sub in range(2):
                    bcast_ap = bass.AP(
                        tensor=recip_dram[:, :].tensor,
                        offset=hp * 2 * HW + sub * HW,
                        ap=[[0, 64], [1, HW]])
                    nc.sync.dma_start(out=rb[sub], in_=bcast_ap)
                # fused normalize + eviction (writes at base 64 are legal as
                # long as both INPUTS share a base)
                nc.vector.tensor_mul(
                    attn_sb[hp][0:64, :], po_pair[0][0:64, :], rb[0][:, :])
                nc.vector.tensor_mul(
                    attn_sb[hp][64:128, :], po_pair[1][0:64, :], rb[1][:, :])
                if debug and hp == 0:
                    nc.gpsimd.dma_start(out=dbg["den0"][0:1, :], in_=den[0:1, :])
                    nc.gpsimd.dma_start(out=dbg["rr0"][0:1, :], in_=rr[0:1, :])
                    nc.gpsimd.dma_start(out=dbg["rb00"][:, :], in_=rb[0])

            # ---------- emission schedule ----------
            emit_qk(0)
            emit_qk(1)
            # pair 0 logits+exp as early as possible (ACT is the wall)
            for p in range(PT):
                emit_logits_exp(0, p)
            for p in range(PT):
                emit_v(p)
            emit_qk(2)
            emit_qk(3)
            pin_cm.__exit__(None, None, None)  # free pin's 4 PSUM banks for po

            with tc.tile_pool(name="po_pool", bufs=2, space="PSUM") as po_pool:
                for hp in range(4):
                    po_pair = [po_pool.tile([HD + 1, HW], F32, tag="po",
                                            name=f"po{2 * hp + sub}")
                               for sub in range(2)]
                    # interleave out2(hp) with logits/exp(hp+1) per p-step;
                    # out2 first so ready PE work isn't blocked in the FIFO.
                    for p in range(PT):
                        emit_out2_mm(hp, p, po_pair)
                        if hp + 1 < 4:
                            emit_logits_exp(hp + 1, p)
                    eT_all.pop(hp)
                    emit_norm(hp, po_pair)

            if debug:
                nc.gpsimd.dma_start(out=dbg["h0"][:, :], in_=h_sb[0])
                nc.gpsimd.dma_start(out=dbg["q0"][:, :], in_=q_sb[0])
                nc.gpsimd.dma_start(out=dbg["k0"][:, :], in_=k_sb[0])
                nc.gpsimd.dma_start(
                    out=dbg["vT0"][:, :],
                    in_=vT_sb[0].rearrange("a nh c -> a (nh c)"))
                nc.gpsimd.dma_start(out=dbg["attn0"][:, :], in_=attn_sb[0])

            # ---------- proj_out + c0 + residual ----------
            with tc.tile_pool(name="pout", bufs=1, space="PSUM") as pout:
                for m in range(4):
                    o_sb = small.tile([128, HW], F32, tag="osb", bufs=2,
                                      name=f"osb{m}")
                    pps = [pout.tile([128, 512], F32, tag="ppo", bufs=4,
                                     name=f"ppo{m}_{n}") for n in range(2)]
                    for t in range(CT):
                        for n in range(2):
                            nc.tensor.matmul(
                                pps[n][:, :],
                                lhsT=w_outT_sb[t][:, 128 * m:128 * (m + 1)],
                                rhs=attn_sb[t][:, 512 * n:512 * (n + 1)],
                                start=(t == 0), stop=(t == CT - 1))
                    for n in range(2):
                        nc.vector.scalar_tensor_tensor(
                            out=o_sb[:, 512 * n:512 * (n + 1)],
                            in0=pps[n][:, :], scalar=c0_sb[:, m:m + 1],
                            in1=x_sb[m][:, 512 * n:512 * (n + 1)],
                            op0=ADD, op1=ADD)
                    nc.sync.dma_start(
                        out=out_ext[128 * m:128 * (m + 1), :], in_=o_sb)
    return nc


def _install_ntff_hook():
    """The agent image's antenv lacks axon_hooks; synthesize it so
    run_bass_kernel_spmd(trace=True) can reach the NTFF profiler."""
    import types
    if "antenv.axon_hooks" in sys.modules:
        return
    mod = types.ModuleType("antenv.axon_hooks")
    mod._hook = None

    def set_axon_ntff_profile_hook(hook):
        mod._hook = hook

    def get_axon_ntff_profile_hook():
        return mod._hook

    mod.set_axon_ntff_profile_hook = set_axon_ntff_profile_hook
    mod.get_axon_ntff_profile_hook = get_axon_ntff_profile_hook
    sys.modules["antenv.axon_hooks"] = mod
    try:
        from trn_agent_boot.trn_boot import _ntff_profile_via_ctypes
        hook = _ntff_profile_via_ctypes("/opt/axon/libaxon_pjrt.so")
        if hook is not None:
            set_axon_ntff_profile_hook(hook)
    except Exception as e:  # degrade to no tracing
        print("ntff hook setup failed:", e)


_COMPILED = None


def _get_compiled():
    global _COMPILED
    if _COMPILED is None:
        nc = build_graph()
        nc.compile()
        _COMPILED = nc
    return _COMPILED


def _make_consts():
    sel = np.zeros((128, 8), dtype=np.float32)
    selT = np.zeros((8, 128), dtype=np.float32)
    for p in range(128):
        g = p // GS
        sel[p, g] = 1.0
        selT[g, p] = 1.0
    return sel, selT


def _pm(v, cols):
    """[cols*128] vector -> partition-major [128, cols]."""
    return np.ascontiguousarray(v.reshape(cols, 128).T)


def kernel(x, gamma, beta, w_in, b_in, w_out, b_out, _trace=False):
    x = np.asarray(x, dtype=np.float32)
    gamma = np.asarray(gamma, dtype=np.float32)
    beta = np.asarray(beta, dtype=np.float32)
    w_in = np.asarray(w_in, dtype=np.float32)
    b_in = np.asarray(b_in, dtype=np.float32)
    w_out = np.asarray(w_out, dtype=np.float32)
    b_out = np.asarray(b_out, dtype=np.float32)

    # fold gamma into w_in columns, beta into the qkv bias
    w_inT_g = np.ascontiguousarray((w_in * gamma[None, :]).T).astype(
        ml_dtypes.bfloat16)
    b_eff = b_in + w_in @ beta
    b_q = b_eff[0:HID]
    b_k = b_eff[HID:2 * HID]
    b_v = b_eff[2 * HID:3 * HID]
    c0 = w_out @ b_v + b_out
    w_outT = np.ascontiguousarray(w_out.T).astype(ml_dtypes.bfloat16)
    sel, selT = _make_consts()
    common = {
        "w_inT_g": w_inT_g,
        "w_outT": w_outT,
        "b_q_pm": _pm(b_q, CT),
        "b_k_pm": _pm(b_k, CT),
        "c0_pm": _pm(c0, CT),
        "gn_sel8": sel,
        "gn_selT8": selT,
    }
    in_maps = []
    for b in range(B):
        m = dict(common)
        m["xbf"] = np.ascontiguousarray(x[b].reshape(C, HW)).astype(
            ml_dtypes.bfloat16)
        in_maps.append(m)

    if _trace:
        _install_ntff_hook()
    nc = _get_compiled()
    res = run_bass_kernel_spmd(nc, in_maps, core_ids=list(range(B)),
                               trace=_trace)
    out = np.stack([np.asarray(res.results[b]["out"]).reshape(C, H, W)
                    for b in range(B)])
    if _trace:
        return out, res
    return out


if __name__ == "__main__":
    rng = np.random.default_rng(0)
    inputs = {
        "x": rng.standard_normal((B, C, H, W), dtype=np.float32),
        "gamma": np.ones(C, dtype=np.float32),
        "beta": np.zeros(C, dtype=np.float32),
        "w_in": (rng.standard_normal((3 * HID, C), dtype=np.float32)
                 / np.sqrt(C)),
        "b_in": np.zeros(3 * HID, dtype=np.float32),
        "w_out": (rng.standard_normal((C, HID), dtype=np.float32)
                  / np.sqrt(HID)),
        "b_out": np.zeros(C, dtype=np.float32),
    }
    out = kernel(**inputs)
    print("kernel ran, out shape", out.shape)


# revision 10
# speedup vs baseline: 1.5992x; 1.1884x over previous
"""Trainium2 Bass kernel for nn_AttentionBlock (GroupNorm + 8-head attention
block on [8, 512, 32, 32], residual).

Sharding: pure data-parallel over batch B=8 across the 8 NeuronCores — one
batch element per core, weights replicated, zero collectives.

v2 design (ACT-exp is the wall at ~73us; everything else hides under it):
  - gamma/beta folded into host-preprocessed weights: w_inT_g = w_in.T * gamma,
    biases b_eff = b_in + w_in @ beta.  Device GN = (x - mean) * rstd only,
    with rstd = exp(-0.5*ln(var+eps)) so the whole kernel uses ONE ACT table
    set (natural_log_exp: ln, exp, square, identity).
  - x shipped as bf16 (halves input DMA); per-channel-tile GN pipelined so
    proj_in matmuls start as soon as h tiles exist.
  - v-bias and out-bias deferred: c0 = w_out @ b_v_eff + b_out added at the
    final residual step (softmax weights sum to 1).
  - PE warm-up junk matmuls at start (HAM clock gate: 1.2 -> 2.4 GHz after
    ~3.4us of sustained busy).
  - attention: q,k projected first, logits+exp of pair 0 launched before the
    v projection; pairs software-pipelined; out2 uses a ones-column (M=65) to
    get softmax denominators for free; denominators evicted per-pair,
    reciprocal_approx_fast, DRAM-round-trip broadcast, normalize fused into
    the PSUM eviction (one DVE tensor_tensor per head).
"""
import sys

sys.path.insert(0, "/opt/trn_rl_repo")

import numpy as np
import ml_dtypes

import concourse.bass as bass
import concourse.bacc as bacc
import concourse.tile as tile
from concourse import mybir
from concourse.bass_utils import run_bass_kernel_spmd

F32 = mybir.dt.float32
BF16 = mybir.dt.bfloat16
ADD = mybir.AluOpType.add
SUB = mybir.AluOpType.subtract
MULT = mybir.AluOpType.mult
AF = mybir.ActivationFunctionType

B, C, H, W = 8, 512, 32, 32
HW = H * W       # 1024
NG = 32          # groups
GS = C // NG     # 16 channels per group
NH = 8           # heads
HD = 64          # head dim
HID = NH * HD    # 512
EPS = 1e-6
SCALE = 1.0 / float(np.sqrt(HD))  # 0.125
CT = C // 128    # 4 channel partition-tiles
PT = HW // 128   # 8 pixel partition-tiles
GN_INV = 1.0 / (GS * HW)          # 1/16384
N_JUNK = 14      # PE warm-up matmuls


def build_graph(debug=False):
    nc = bacc.Bacc("TRN2", num_devices=8)

    x_ext = nc.declare_dram_parameter("xbf", [C, HW], BF16, isOutput=False)
    w_inT_ext = nc.declare_dram_parameter("w_inT_g", [C, 3 * HID], BF16, isOutput=False)
    w_outT_ext = nc.declare_dram_parameter("w_outT", [HID, C], BF16, isOutput=False)
    b_q_ext = nc.declare_dram_parameter("b_q_pm", [128, CT], F32, isOutput=False)
    b_k_ext = nc.declare_dram_parameter("b_k_pm", [128, CT], F32, isOutput=False)
    c0_ext = nc.declare_dram_parameter("c0_pm", [128, CT], F32, isOutput=False)
    sel_ext = nc.declare_dram_parameter("gn_sel8", [128, 8], F32, isOutput=False)
    selT_ext = nc.declare_dram_parameter("gn_selT8", [8, 128], F32, isOutput=False)
    out_ext = nc.declare_dram_parameter("out", [C, HW], F32, isOutput=True)

    recip_dram = nc.dram_tensor("recip_scratch", [4, 2 * HW], F32)
    junk_dram = nc.dram_tensor("junk_scratch", [1, 512], F32)
    dbg = {}
    if debug:
        dbg["h0"] = nc.declare_dram_parameter("dbg_h0", [128, HW], BF16, isOutput=True)
        dbg["q0"] = nc.declare_dram_parameter("dbg_q0", [128, HW], BF16, isOutput=True)
        dbg["k0"] = nc.declare_dram_parameter("dbg_k0", [128, HW], BF16, isOutput=True)
        dbg["vT0"] = nc.declare_dram_parameter("dbg_vT0", [128, NH * (HD + 1)], BF16, isOutput=True)
        dbg["eT000"] = nc.declare_dram_parameter("dbg_eT000", [128, HW], BF16, isOutput=True)
        dbg["den0"] = nc.declare_dram_parameter("dbg_den0", [1, 2 * HW], F32, isOutput=True)
        dbg["rr0"] = nc.declare_dram_parameter("dbg_rr0", [1, 2 * HW], F32, isOutput=True)
        dbg["rb00"] = nc.declare_dram_parameter("dbg_rb00", [64, HW], F32, isOutput=True)
        dbg["attn0"] = nc.declare_dram_parameter("dbg_attn0", [128, HW], BF16, isOutput=True)

    with tile.TileContext(nc) as tc:
        with (
            tc.tile_pool(name="const", bufs=1) as const,
            tc.tile_pool(name="big", bufs=1) as big,
            tc.tile_pool(name="eT", bufs=1) as eTp,
            tc.tile_pool(name="small", bufs=2) as small,
            tc.tile_pool(name="pl_pool", bufs=2, space="PSUM") as pl_pool,
        ):
            pin_cm = tc.tile_pool(name="pin", bufs=1, space="PSUM")
            pin = pin_cm.__enter__()
            # ---------- tiny on-chip constants (no DMA) ----------
            warm_sb = small.tile([128, 512], BF16, tag="warm", bufs=1)
            nc.vector.memset(warm_sb, 0.25)
            eps_sb = small.tile([8, 1], F32, tag="eps_c", bufs=1)
            nc.vector.memset(eps_sb, float(EPS))
            # preload the natural_log_exp act table set ASAP
            dummy_sb = small.tile([1, 1], F32, tag="dummy", bufs=1)
            nc.scalar.activation(out=dummy_sb, in_=eps_sb[0:1, 0:1], func=AF.Ln,
                                 bias=eps_sb[0:1, :], scale=1.0)

            # ---------- input DMAs, issue spread across engines ----------
            x_sb = [big.tile([128, HW], BF16, tag=f"x{t}", name=f"x{t}")
                    for t in range(CT)]
            for t in range(CT):
                nc.sync.dma_start(out=x_sb[t], in_=x_ext[128 * t:128 * (t + 1), :])
            sel_sb = const.tile([128, 8], F32)
            nc.gpsimd.dma_start(out=sel_sb, in_=sel_ext[:, :])
            selT_sb = const.tile([8, 128], F32)
            nc.gpsimd.dma_start(out=selT_sb, in_=selT_ext[:, :])
            w_inT_sb = [big.tile([128, 3 * HID], BF16, tag=f"wi{t}", name=f"wi{t}")
                        for t in range(CT)]
            for t in range(CT):
                nc.gpsimd.dma_start(out=w_inT_sb[t],
                                    in_=w_inT_ext[128 * t:128 * (t + 1), :])
            b_q_sb = const.tile([128, CT], F32)
            nc.gpsimd.dma_start(out=b_q_sb, in_=b_q_ext[:, :])
            b_k_sb = const.tile([128, CT], F32)
            nc.gpsimd.dma_start(out=b_k_sb, in_=b_k_ext[:, :])
            c0_sb = const.tile([128, CT], F32)
            nc.gpsimd.dma_start(out=c0_sb, in_=c0_ext[:, :])
            w_outT_sb = [big.tile([128, C], BF16, tag=f"wo{t}", name=f"wo{t}")
                         for t in range(CT)]
            for t in range(CT):
                nc.sync.dma_start(out=w_outT_sb[t],
                                  in_=w_outT_ext[128 * t:128 * (t + 1), :])

            # ---------- PE warm-up (HAM un-throttle) ----------
            jp = None
            for j in range(N_JUNK):
                jp = pin.tile([128, 512], F32, tag="pp", bufs=2, name=f"junk{j}")
                nc.tensor.matmul(jp[:, :], lhsT=warm_sb[:, 0:128],
                                 rhs=warm_sb[:, :], start=True, stop=True)
            junk_sb = small.tile([1, 512], F32, tag="junk_s", bufs=1)
            nc.vector.tensor_copy(out=junk_sb, in_=jp[0:1, :])
            nc.sync.dma_start(out=junk_dram[0:1, :], in_=junk_sb)

            # ---------- groupnorm (per 128-channel tile; groups don't cross
            # tiles).  h[t] = x[t]*rstd - mean*rstd, gamma/beta in weights.
            # ln/exp batched across tiles: exactly one act-table switch. ----
            h_sb = [big.tile([128, HW], BF16, tag=f"h{t}", name=f"h{t}")
                    for t in range(CT)]
            sq_scratch = [small.tile([128, HW], BF16, tag=f"sqs{t % 2}", bufs=1,
                                     name=f"sqs{t}") for t in range(CT)]
            # var_all[:, t] = group variances of tile t; rstd_all = exp(-ln/2)
            var_all = small.tile([8, CT], F32, tag="var_all", bufs=1)
            lnv_all = small.tile([8, CT], F32, tag="lnv_all", bufs=1)
            rstd_all = small.tile([8, CT], F32, tag="rstd_all", bufs=1)
            mean_all = small.tile([8, CT], F32, tag="mean_all", bufs=1)
            grp2 = [small.tile([8, 2], F32, tag=f"grp2_{t}", bufs=1,
                               name=f"grp2_{t}") for t in range(CT)]
            for t in range(CT):
                stats = small.tile([128, 2], F32, tag=f"st{t}", bufs=1,
                                   name=f"st{t}")
                nc.vector.reduce_sum(stats[:, 0:1], x_sb[t][:, :],
                                     axis=mybir.AxisListType.X)
                nc.scalar.activation(out=sq_scratch[t], in_=x_sb[t][:, :],
                                     func=AF.Square,
                                     accum_out=stats[:, 1:2])
                gpsum = pin.tile([8, 2], F32, tag="gps", bufs=1, name=f"gps{t}")
                nc.tensor.matmul(gpsum[:, :], lhsT=sel_sb[:, :],
                                 rhs=stats[:, :], start=True, stop=True)
                ms = small.tile([8, 1], F32, tag=f"ms{t}", bufs=1,
                                name=f"ms{t}")
                nc.vector.tensor_scalar_mul(mean_all[:, t:t + 1],
                                            gpsum[:, 0:1], GN_INV)
                nc.vector.tensor_scalar_mul(ms, gpsum[:, 1:2], GN_INV)
                nc.vector.tensor_mul(var_all[:, t:t + 1],
                                     mean_all[:, t:t + 1],
                                     mean_all[:, t:t + 1])
                nc.vector.tensor_sub(var_all[:, t:t + 1], ms[:, :],
                                     var_all[:, t:t + 1])
            nc.scalar.activation(out=lnv_all, in_=var_all, func=AF.Ln,
                                 bias=eps_sb[:, :], scale=1.0)
            nc.scalar.activation(out=rstd_all, in_=lnv_all, func=AF.Exp,
                                 scale=-0.5)
            for t in range(CT):
                nc.vector.tensor_copy(out=grp2[t][:, 0:1],
                                      in_=rstd_all[:, t:t + 1])
                nc.vector.tensor_mul(grp2[t][:, 1:2], mean_all[:, t:t + 1],
                                     rstd_all[:, t:t + 1])
                epsum = pin.tile([128, 2], F32, tag="eps_ps", bufs=1,
                                 name=f"eps_ps{t}")
                nc.tensor.matmul(epsum[:, :], lhsT=selT_sb[:, :],
                                 rhs=grp2[t][:, :], start=True, stop=True)
                ab = small.tile([128, 2], F32, tag=f"ab{t}", bufs=1,
                                name=f"ab{t}")
                nc.vector.tensor_copy(out=ab, in_=epsum[:, :])
                nc.vector.tensor_scalar(
                    out=h_sb[t], in0=x_sb[t][:, :],
                    scalar1=ab[:, 0:1], scalar2=ab[:, 1:2],
                    op0=MULT, op1=SUB)

            # ---------- proj_in q,k (m=0 first), then logits pair0, then v --
            q_sb = [big.tile([128, HW], BF16, tag=f"q{m}", name=f"q{m}")
                    for m in range(4)]
            k_sb = [big.tile([128, HW], BF16, tag=f"k{m}", name=f"k{m}")
                    for m in range(4)]
            vT_sb = [big.tile([128, NH, HD + 1], BF16, tag=f"vT{p}",
                              name=f"vT{p}") for p in range(PT)]
            for p in range(PT):
                nc.gpsimd.memset(vT_sb[p], 1.0)

            def emit_qk(m):
                for dest, off, bias in ((q_sb, 0, b_q_sb), (k_sb, HID, b_k_sb)):
                    pps = [pin.tile([128, 512], F32, tag="pp", bufs=2,
                                    name=f"pp{'qk'[off > 0]}{m}_{n}")
                           for n in range(2)]
                    for t in range(CT):
                        for n in range(2):
                            nc.tensor.matmul(
                                pps[n][:, :],
                                lhsT=w_inT_sb[t][:, off + 128 * m:
                                                 off + 128 * (m + 1)],
                                rhs=h_sb[t][:, 512 * n:512 * (n + 1)],
                                start=(t == 0), stop=(t == CT - 1))
                    for n in range(2):
                        nc.vector.tensor_scalar(
                            out=dest[m][:, 512 * n:512 * (n + 1)],
                            in0=pps[n][:, :],
                            scalar1=bias[:, m:m + 1], scalar2=None, op0=ADD)

            def emit_v(p):
                pp = pin.tile([128, 512], F32, tag="pp", bufs=2, name=f"ppv{p}")
                for t in range(CT):
                    nc.tensor.matmul(
                        pp[:, :],
                        lhsT=h_sb[t][:, 128 * p:128 * (p + 1)],
                        rhs=w_inT_sb[t][:, 2 * HID:3 * HID],
                        start=(t == 0), stop=(t == CT - 1))
                nc.vector.tensor_copy(
                    out=vT_sb[p][:, :, 0:HD],
                    in_=pp[:, :].rearrange("a (nh c) -> a nh c", nh=NH))

            # attention helpers -------------------------------------------
            attn_sb = [big.tile([128, HW], BF16, tag=f"at{i}", name=f"at{i}")
                       for i in range(4)]
            eT_all = {}

            def emit_logits_exp(hp, p):
                """logits + exp for pair hp, pixel-tile p (2 heads row-paired)."""
                if p == 0:
                    eT_all[hp] = [[eTp.tile([128, HW], BF16, bufs=2,
                                            tag=f"eT{sub}_{pp_}",
                                            name=f"eT{hp}_{sub}_{pp_}")
                                   for pp_ in range(PT)] for sub in range(2)]
                pls = []
                for sub in range(2):
                    pls.append(pl_pool.tile([128, HW], F32, tag="pl",
                                            name=f"pl{hp}_{sub}_{p}"))
                for n in range(2):
                    for sub in range(2):
                        lo, hi = 64 * sub, 64 * (sub + 1)
                        nc.tensor.matmul(
                            pls[sub][:, 512 * n:512 * (n + 1)],
                            lhsT=k_sb[hp][lo:hi, 128 * p:128 * (p + 1)],
                            rhs=q_sb[hp][lo:hi, 512 * n:512 * (n + 1)],
                            start=True, stop=True)
                for sub in range(2):
                    nc.scalar.activation(
                        out=eT_all[hp][sub][p], in_=pls[sub][:, :],
                        func=AF.Exp, scale=SCALE)
                if debug and hp == 0 and p == 0:
                    nc.gpsimd.dma_start(out=dbg["eT000"][:, :],
                                        in_=eT_all[0][0][0])

            def emit_out2_mm(hp, p, po_pair):
                """out2 accumulation step p for both heads of pair hp."""
                for sub in range(2):
                    head = 2 * hp + sub
                    for n in range(2):
                        nc.tensor.matmul(
                            po_pair[sub][:, 512 * n:512 * (n + 1)],
                            lhsT=vT_sb[p][:, head, :],
                            rhs=eT_all[hp][sub][p][:, 512 * n:512 * (n + 1)],
                            start=(p == 0), stop=(p == PT - 1))

            def emit_norm(hp, po_pair):
                """Evict u+denominator (frees PSUM fast), then decoupled
                recip + broadcast + normalize (attn only needed by proj_out).
                reciprocal_approx_fast only works at partition base 0."""
                uden = [small.tile([65, HW], BF16, tag=f"uden{sub}", bufs=2,
                                   name=f"uden{hp}_{sub}") for sub in range(2)]
                for sub in range(2):
                    nc.vector.tensor_copy(out=uden[sub], in_=po_pair[sub][:, :])
                den = small.tile([1, 2 * HW], F32, tag="den", bufs=1,
                                 name=f"den{hp}")
                for sub in range(2):
                    nc.vector.tensor_copy(
                        out=den[0:1, HW * sub:HW * (sub + 1)],
                        in_=uden[sub][64:65, :])
                rr = small.tile([1, 2 * HW], F32, tag="rr", bufs=1,
                                name=f"rr{hp}")
                nc.vector.reciprocal_approx_fast(out=rr, in_=den)
                nc.sync.dma_start(out=recip_dram[hp:hp + 1, :], in_=rr)
                rb = [small.tile([64, HW], F32, tag=f"rb{sub}", bufs=2,
                                 name=f"rb{hp}_{sub}") for sub in range(2)]
                for sub in range(2):
                    bcast_ap = bass.AP(
                        tensor=recip_dram[:, :].tensor,
                        offset=hp * 2 * HW + sub * HW,
                        ap=[[0, 64], [1, HW]])
                    nc.sync.dma_start(out=rb[sub], in_=bcast_ap)
                nc.vector.tensor_mul(
                    attn_sb[hp][0:64, :], uden[0][0:64, :], rb[0][:, :])
                nc.vector.tensor_mul(
                    attn_sb[hp][64:128, :], uden[1][0:64, :], rb[1][:, :])

            # ---------- emission schedule ----------
            emit_qk(0)
            emit_qk(1)
            # pair 0 logits/exp trickle at exp pace (pl slots); interleave
            # independent PE work so the PE FIFO never stalls behind them.
            emit_logits_exp(0, 0)
            emit_logits_exp(0, 1)
            emit_v(0)
            emit_logits_exp(0, 2)
            emit_v(1)
            emit_logits_exp(0, 3)
            emit_qk(2)
            emit_logits_exp(0, 4)
            emit_v(2)
            emit_logits_exp(0, 5)
            emit_qk(3)
            emit_logits_exp(0, 6)
            for p in range(3, PT):
                emit_v(p)
            emit_logits_exp(0, 7)
            pin_cm.__exit__(None, None, None)  # free pin's 4 PSUM banks for po

            with tc.tile_pool(name="po_pool", bufs=2, space="PSUM") as po_pool:
                for hp in range(4):
                    po_pair = [po_pool.tile([HD + 1, HW], F32, tag="po",
                                            name=f"po{2 * hp + sub}")
                               for sub in range(2)]
                    # interleave out2(hp) with logits/exp(hp+1) per p-step;
                    # out2 first so ready PE work isn't blocked in the FIFO.
                    for p in range(PT):
                        emit_out2_mm(hp, p, po_pair)
                        if hp + 1 < 4:
                            emit_logits_exp(hp + 1, p)
                    eT_all.pop(hp)
                    emit_norm(hp, po_pair)

            if debug:
                nc.gpsimd.dma_start(out=dbg["h0"][:, :], in_=h_sb[0])
                nc.gpsimd.dma_start(out=dbg["q0"][:, :], in_=q_sb[0])
                nc.gpsimd.dma_start(out=dbg["k0"][:, :], in_=k_sb[0])
                nc.gpsimd.dma_start(
                    out=dbg["vT0"][:, :],
                    in_=vT_sb[0].rearrange("a nh c -> a (nh c)"))
                nc.gpsimd.dma_start(out=dbg["attn0"][:, :], in_=attn_sb[0])

            # ---------- proj_out + c0 + residual ----------
            # n-outer/t-outer: the 24 matmuls on attn chunks 0..2 only wait
            # for PSUM banks (freed by the last exps), overlapping the final
            # pair's normalize chain.
            with tc.tile_pool(name="pout", bufs=1, space="PSUM") as pout:
                o_sb = [small.tile([128, HW], F32, tag="osb", bufs=4,
                                   name=f"osb{m}") for m in range(4)]
                for n in range(2):
                    pps = [pout.tile([128, 512], F32, tag="ppo", bufs=4,
                                     name=f"ppo{m}_{n}") for m in range(4)]
                    for t in range(CT):
                        for m in range(4):
                            nc.tensor.matmul(
                                pps[m][:, :],
                                lhsT=w_outT_sb[t][:, 128 * m:128 * (m + 1)],
                                rhs=attn_sb[t][:, 512 * n:512 * (n + 1)],
                                start=(t == 0), stop=(t == CT - 1))
                    for m in range(4):
                        nc.vector.scalar_tensor_tensor(
                            out=o_sb[m][:, 512 * n:512 * (n + 1)],
                            in0=pps[m][:, :], scalar=c0_sb[:, m:m + 1],
                            in1=x_sb[m][:, 512 * n:512 * (n + 1)],
                            op0=ADD, op1=ADD)
                        if n == 1:
                            nc.sync.dma_start(
                                out=out_ext[128 * m:128 * (m + 1), :],
                                in_=o_sb[m])
    return nc


def _install_ntff_hook():
    """The agent image's antenv lacks axon_hooks; synthesize it so
    run_bass_kernel_spmd(trace=True) can reach the NTFF profiler."""
    import types
    if "antenv.axon_hooks" in sys.modules:
        return
    mod = types.ModuleType("antenv.axon_hooks")
    mod._hook = None

    def set_axon_ntff_profile_hook(hook):
        mod._hook = hook

    def get_axon_ntff_profile_hook():
        return mod._hook

    mod.set_axon_ntff_profile_hook = set_axon_ntff_profile_hook
    mod.get_axon_ntff_profile_hook = get_axon_ntff_profile_hook
    sys.modules["antenv.axon_hooks"] = mod
    try:
        from trn_agent_boot.trn_boot import _ntff_profile_via_ctypes
        hook = _ntff_profile_via_ctypes("/opt/axon/libaxon_pjrt.so")
        if hook is not None:
            set_axon_ntff_profile_hook(hook)
    except Exception as e:  # degrade to no tracing
        print("ntff hook setup failed:", e)


_COMPILED = None


def _get_compiled():
    global _COMPILED
    if _COMPILED is None:
        nc = build_graph()
        nc.compile()
        _COMPILED = nc
    return _COMPILED


def _make_consts():
    sel = np.zeros((128, 8), dtype=np.float32)
    selT = np.zeros((8, 128), dtype=np.float32)
    for p in range(128):
        g = p // GS
        sel[p, g] = 1.0
        selT[g, p] = 1.0
    return sel, selT


def _pm(v, cols):
    """[cols*128] vector -> partition-major [128, cols]."""
    return np.ascontiguousarray(v.reshape(cols, 128).T)


def kernel(x, gamma, beta, w_in, b_in, w_out, b_out, _trace=False):
    x = np.asarray(x, dtype=np.float32)
    gamma = np.asarray(gamma, dtype=np.float32)
    beta = np.asarray(beta, dtype=np.float32)
    w_in = np.asarray(w_in, dtype=np.float32)
    b_in = np.asarray(b_in, dtype=np.float32)
    w_out = np.asarray(w_out, dtype=np.float32)
    b_out = np.asarray(b_out, dtype=np.float32)

    # fold gamma into w_in columns, beta into the qkv bias
    w_inT_g = np.ascontiguousarray((w_in * gamma[None, :]).T).astype(
        ml_dtypes.bfloat16)
    b_eff = b_in + w_in @ beta
    b_q = b_eff[0:HID]
    b_k = b_eff[HID:2 * HID]
    b_v = b_eff[2 * HID:3 * HID]
    c0 = w_out @ b_v + b_out
    w_outT = np.ascontiguousarray(w_out.T).astype(ml_dtypes.bfloat16)
    sel, selT = _make_consts()
    common = {
        "w_inT_g": w_inT_g,
        "w_outT": w_outT,
        "b_q_pm": _pm(b_q, CT),
        "b_k_pm": _pm(b_k, CT),
        "c0_pm": _pm(c0, CT),
        "gn_sel8": sel,
        "gn_selT8": selT,
    }
    in_maps = []
    for b in range(B):
        m = dict(common)
        m["xbf"] = np.ascontiguousarray(x[b].reshape(C, HW)).astype(
            ml_dtypes.bfloat16)
        in_maps.append(m)

    if _trace:
        _install_ntff_hook()
    nc = _get_compiled()
    res = run_bass_kernel_spmd(nc, in_maps, core_ids=list(range(B)),
                               trace=_trace)
    out = np.stack([np.asarray(res.results[b]["out"]).reshape(C, H, W)
                    for b in range(B)])
    if _trace:
        return out, res
    return out


if __name__ == "__main__":
    rng = np.random.default_rng(0)
    inputs = {
        "x": rng.standard_normal((B, C, H, W), dtype=np.float32),
        "gamma": np.ones(C, dtype=np.float32),
        "beta": np.zeros(C, dtype=np.float32),
        "w_in": (rng.standard_normal((3 * HID, C), dtype=np.float32)
                 / np.sqrt(C)),
        "b_in": np.zeros(3 * HID, dtype=np.float32),
        "w_out": (rng.standard_normal((C, HID), dtype=np.float32)
                  / np.sqrt(HID)),
        "b_out": np.zeros(C, dtype=np.float32),
    }
    out = kernel(**inputs)
    print("kernel ran, out shape", out.shape)
